# revision 35
# baseline (speedup 1.0000x reference)
"""BiMamba block Trainium2 kernel (8 NeuronCores, communication-free sharding).

Sharding: 8 cores = 2 directions x 2 batches x 2 head-halves (12 of 24 Mamba2
heads per core).  Per core: in_proj slice -> causal depthwise conv (diagonal
matmuls) -> chunked SSD scan (chunk=128) -> gating -> partial out-projection
with the merged (out_proj @ inner_out_proj * norm_w) weight.  The gated
RMSNorm's row scaling commutes with the final matmul, so each core returns an
unnormalized partial [768, 512] plus a per-token sum-of-squares row; the host
applies rsqrt(mean+eps), sums partials, reverses the backward direction and
adds the residual.  No inter-core communication.

The dt -> softplus -> cumsum -> exp decay math (0.06 GFLOP) is precomputed on
the host in f64 and shipped as bf16 decay masks; all device matmuls are bf16
with f32 PSUM accumulation.

Perf notes (vs the first working version):
 - inputs are packed into a handful of DRAM params so the DMA queue issues
   ~12 large transfers instead of 40 small ones
 - all 24 x^T chunk tiles are produced ahead of the scan: chunk 0 via PE
   transposes (lowest latency), chunks 1-3 via DMA transposes split across
   the two HWDGE queues (sync + scalar), all overlapped with phase 1
 - decay masks / chunk-state scalings for every chunk are prebuilt before
   the scan so the scan's critical path is matmuls + gating only
 - the conv processes the B/C tile first so the G2 masks and B^T tiles are
   ready early
 - g^2 for the row sum-of-squares runs on DVE, not the Activation engine
 - a dummy Silu at kernel start pre-loads the activation table
"""

import sys

sys.path.insert(0, "/opt/trn_rl_repo")

import ml_dtypes
import numpy as np

import concourse.bacc as bacc
import concourse.bass as bass
import concourse.mybir as mybir
from concourse.tile import TileContext

FP = mybir.dt.float32
BF = mybir.dt.bfloat16
NPBF = ml_dtypes.bfloat16

D_MODEL = 768
D_STATE = 32
D_CONV = 4
D_INNER = 1536
HEADDIM = 64
CONV_DIM = D_INNER + 2 * D_STATE  # 1600
B_SZ, SEQ = 2, 512
EPS = 1e-5

H = 12                      # heads per core
DI = H * HEADDIM            # 768 d_inner slice per core
XBC = DI + 2 * D_STATE      # 832 conv channels per core
NCT = 7                     # conv channel tiles (6x128 + 1x64)
LC = 128                    # chunk length
NCHUNK = SEQ // LC          # 4
KT = D_MODEL // 128         # 6 k tiles
IT = DI // 128              # 6 d_inner tiles per core
OT = D_MODEL // 128         # 6 output tiles

PKA = SEQ + XBC             # 1344 packed cols per k tile (uT | wxbcT)

AF = mybir.ActivationFunctionType
OP = mybir.AluOpType


def build_nc():
    nc = bacc.Bacc(target_bir_lowering=False)

    hd_d = nc.declare_dram_parameter("hd", [128, 224], BF, isOutput=False)

    pka_d = nc.declare_dram_parameter("pka", [KT, 128, PKA], BF, isOutput=False)
    pkb_d = nc.declare_dram_parameter("pkb", [KT, 128, DI], BF, isOutput=False)
    me_d = nc.declare_dram_parameter("me", [128, (NCHUNK + 1) * H * 128], BF, isOutput=False)
    esc_d = nc.declare_dram_parameter("esc", [32, (NCHUNK - 1) * H * 128], BF, isOutput=False)
    wm_d = nc.declare_dram_parameter("wm", [128, KT * D_MODEL], BF, isOutput=False)
    out_d = nc.declare_dram_parameter("out", [D_MODEL, SEQ], BF, isOutput=True)
    oss_d = nc.declare_dram_parameter("oss", [1, SEQ], FP, isOutput=True)

    ts = bass.ts

    with TileContext(nc) as tc:
        with (
            tc.tile_pool(name="wp", bufs=1) as wp,        # weights + consts
            tc.tile_pool(name="sb", bufs=1) as sbp,       # long-lived activations
        ):
            # activation-table warmup: a dummy Silu with no data deps so the
            # ACT table load overlaps the input DMAs instead of blocking conv
            warm = wp.tile([1, 3], FP, name="warm")
            nc.vector.memset(warm[:, 0:1], 0.0)
            nc.scalar.activation(warm[:, 1:2], warm[:, 0:1], AF.Silu)
            nc.scalar.activation(warm[:, 2:3], warm[:, 0:1], AF.Square)

            pka0 = wp.tile_from(pka_d[0, :, :], name="pka0")
            hdr = wp.tile_from(hd_d[:, :], name="hdr")
            identb = hdr[:, 0:128]
            onescolb = hdr[:, 128:129]
            # tensor_scalar / activation-bias operands must be f32: one cast
            cwf = wp.tile([128, 35], FP, name="cwf")
            nc.vector.tensor_copy(cwf[:, :], hdr[:, 129:164])
            convbs = [cwf[:, ct:ct + 1] for ct in range(NCT)]
            convws = [cwf[:, 7 + ct * D_CONV:7 + (ct + 1) * D_CONV] for ct in range(NCT)]
            wts = [hdr[:, 164 + c * H:164 + (c + 1) * H] for c in range(NCHUNK - 1)]
            esls = [hdr[0:32, 200 + (c - 1) * H:200 + c * H] for c in range(1, NCHUNK - 1)]

            pkas = [pka0] + [wp.tile_from(pka_d[k, :, :], name=f"pka{k}") for k in range(1, KT)]
            pkbs = [wp.tile_from(pkb_d[k, :, :], name=f"pkb{k}") for k in range(KT)]
            uTs = [pkas[k][:, 0:SEQ] for k in range(KT)]
            wxbcTs = [pkas[k][:, SEQ:SEQ + XBC] for k in range(KT)]
            wzTs = [pkbs[k][:, :] for k in range(KT)]

            me_t = wp.tile_from(me_d[:, :], name="me")
            dmask = me_t[:, NCHUNK * H * 128:]
            esc_t = wp.tile_from(esc_d[:, :], name="esc")
            wm_t = wp.tile_from(wm_d[:, :], name="wm")
            mes = [me_t[:, c * H * 128:(c + 1) * H * 128] for c in range(NCHUNK)]
            escs = [esc_t[:, (c - 1) * H * 128:c * H * 128] for c in range(1, NCHUNK)]
            wmTs = [wm_t[:, k * D_MODEL:(k + 1) * D_MODEL] for k in range(KT)]

            # conv diagonal matrices, built on DVE (Pool stalls DVE via the
            # shared SBUF ports, so it gets no tensor work)
            convds = []
            for ct in range(NCT):
                P = 128 if ct < NCT - 1 else 64
                row = []
                for k in range(D_CONV):
                    cd = wp.tile([128, 128], BF, name=f"cd{ct}_{k}")
                    nc.vector.tensor_scalar(
                        cd[:P, :P], identb[:P, :P], convws[ct][:P, k:k + 1], None, OP.mult
                    )
                    row.append(cd)
                convds.append(row)

            # long-lived SBUF activations
            zs = [sbp.tile([128, SEQ], BF, name=f"zs{i}") for i in range(IT)]
            xc = [sbp.tile([128, SEQ], BF, name=f"xc{i}") for i in range(NCT - 1)]
            bct = sbp.tile([64, SEQ], BF, name="bct")      # conv'd B(0:32) C(32:64)
            ct_sb = sbp.tile([32, SEQ], BF, name="ct_sb")  # C rows re-based to partition 0
            g = [sbp.tile([128, SEQ], BF, name=f"g{i}") for i in range(IT)]
            g2s = [sbp.tile([128, SEQ], BF, name=f"gg{i}") for i in range(IT)]
            bts = [sbp.tile([128, 32], BF, name=f"bt_{c}") for c in range(NCHUNK - 1)]
            g2cs = [sbp.tile([128, 128], BF, name=f"g2c_{c}") for c in range(NCHUNK)]
            # x^T tiles for the scan: xh[c][it] = transpose(xc[it][:, chunk c])
            xhs = [[sbp.tile([128, 128], BF, name=f"xh{c}_{i}") for i in range(IT)]
                   for c in range(NCHUNK)]
            # prebuilt scan tensors
            mask_cs = [sbp.tile([128, H * 128], BF, name=f"mask{c}") for c in range(NCHUNK)]
            cs_cs = [sbp.tile([32, H * 128], BF, name=f"cs{c}") for c in range(1, NCHUNK)]
            bw_cs = [sbp.tile([128, H * 32], BF, name=f"bw{c}") for c in range(NCHUNK - 1)]

            # ---------------- phase 1: in_proj + conv ----------------
            with (
                tc.tile_pool(name="pbig", bufs=2, space="PSUM") as pbig,
                tc.tile_pool(name="psmall", bufs=3, space="PSUM") as psmall,
            ):
                cin = [None] * NCT
                job_groups = (
                    [[("x", 0), ("x", 1), ("x", 2), ("x", 3), ("x", 6)],
                     [("x", 4), ("x", 5)]]
                    + [[("z", i) for i in range(5)], [("z", 5)]]
                )

                def do_conv():
                    # B/C tile (6) first: it unblocks G2 / B^T / cs_c early
                    for ctile in [NCT - 1] + list(range(NCT - 1)):
                        P = 128 if ctile < NCT - 1 else 64
                        pc = pbig.tile([128, SEQ], FP, space="PSUM", name="pc", tag="big", bufs=5)
                        for k in range(D_CONV):
                            nc.tensor.matmul(
                                pc[:P, :], convds[ctile][k][:P, :P], cin[ctile][:P, k:k + SEQ],
                                start=(k == 0), stop=(k == D_CONV - 1),
                            )
                        dst = xc[ctile][:, :] if ctile < NCT - 1 else bct[:, :]
                        nc.scalar.activation(dst, pc[:P, :], AF.Silu, bias=convbs[ctile][:P, :])
                        if ctile == NCT - 1:
                            nc.scalar.copy(ct_sb[:, :], bct[32:64, :])
                            # G2 masks (shared across heads): (B C^T) * causal
                            for c in range(NCHUNK):
                                pg = psmall.tile([128, 128], FP, space="PSUM", name="pg", tag="sm")
                                nc.tensor.matmul(
                                    pg[:, :], bct[0:32, ts(c, 128)], ct_sb[:, ts(c, 128)],
                                    start=True, stop=True,
                                )
                                nc.vector.tensor_copy(g2cs[c][:, :], pg[:, :])
                            # B^T per chunk: [32, 128] -> [128, 32]
                            for c in range(NCHUNK - 1):
                                pbt = psmall.tile([128, 32], BF, space="PSUM", name="pbt", tag="sm")
                                nc.tensor.transpose(pbt[:, :], bct[0:32, ts(c, 128)], identb[0:32, 0:32])
                                nc.vector.tensor_copy(bts[c][:, :], pbt[:, :])
                        else:
                            # x^T tiles for this conv tile, all via PE
                            # transposes (the DMA queues are busy streaming
                            # inputs; PE transposes are ~150ns each)
                            it = ctile
                            for c in range(NCHUNK):
                                ptx = psmall.tile([128, 128], BF, space="PSUM", name="ptx", tag="sm")
                                nc.tensor.transpose(ptx[:, :], xc[it][:, ts(c, 128)], identb[:, :])
                                if (it + c) % 2 == 0:
                                    nc.vector.tensor_copy(xhs[c][it][:, :], ptx[:, :])
                                else:
                                    nc.scalar.copy(xhs[c][it][:, :], ptx[:, :])

                for gi, grp in enumerate(job_groups):
                    if gi == 2:
                        do_conv()
                    ptiles = {}
                    for kind, idx in grp:
                        ptiles[(kind, idx)] = pbig.tile(
                            [128, SEQ], FP, space="PSUM", name="px", tag="big", bufs=5
                        )
                    for k in range(KT):
                        for kind, idx in grp:
                            if kind == "x":
                                P = 128 if idx < NCT - 1 else 64
                                nc.tensor.matmul(
                                    ptiles[(kind, idx)][:P, :],
                                    wxbcTs[k][:, idx * 128:idx * 128 + P], uTs[k][:, :],
                                    start=(k == 0), stop=(k == KT - 1),
                                )
                            else:
                                nc.tensor.matmul(
                                    ptiles[(kind, idx)][:, :],
                                    wzTs[k][:, ts(idx, 128)], uTs[k][:, :],
                                    start=(k == 0), stop=(k == KT - 1),
                                )
                    for kind, idx in grp:
                        if kind == "x":
                            P = 128 if idx < NCT - 1 else 64
                            ci = sbp.tile([128, D_CONV - 1 + SEQ], BF, name=f"cin{idx}")
                            nc.vector.memset(ci[:P, 0:D_CONV - 1], 0.0)
                            nc.vector.tensor_copy(ci[:P, D_CONV - 1:], ptiles[(kind, idx)][:P, :])
                            cin[idx] = ci
                        else:
                            nc.scalar.activation(
                                zs[idx][:, :], ptiles[(kind, idx)][:, :], AF.Silu
                            )

                # prebuild every chunk's scan tensors (DVE) so the scan loop
                # is matmuls + gating only
                for c in range(NCHUNK):
                    nc.vector.tensor_tensor(
                        mask_cs[c][:, :].rearrange("p (h t) -> p h t", h=H),
                        mes[c].rearrange("p (h t) -> p h t", h=H),
                        g2cs[c][:, None, :].to_broadcast([128, H, 128]),
                        OP.mult,
                    )
                    nc.vector.tensor_tensor(mask_cs[c][:, :], mask_cs[c][:, :], dmask[:, :], OP.add)
                    if c > 0:
                        nc.vector.tensor_tensor(
                            cs_cs[c - 1][:, :].rearrange("p (h t) -> p h t", h=H),
                            escs[c - 1].rearrange("p (h t) -> p h t", h=H),
                            ct_sb[:, None, ts(c, 128)].to_broadcast([32, H, 128]),
                            OP.mult,
                        )
                    if c < NCHUNK - 1:
                        nc.vector.tensor_tensor(
                            bw_cs[c][:, :].rearrange("p (h n) -> p h n", h=H),
                            bts[c][:, None, :].to_broadcast([128, H, 32]),
                            wts[c][:, :, None].to_broadcast([128, H, 32]),
                            OP.mult,
                        )

            # ---------------- phase 2: chunked scan ----------------
            with (
                tc.tile_pool(name="py", bufs=4, space="PSUM") as py,
                tc.tile_pool(name="ps", bufs=1, space="PSUM") as psst,
                tc.tile_pool(name="mp", bufs=3) as mp,
            ):
                hprev = None
                for c in range(NCHUNK):
                    hprev_old = hprev
                    last = c == NCHUNK - 1

                    if not last:
                        s_half = [psst.tile([32, 384], FP, space="PSUM", name=f"sh{j}") for j in range(2)]
                        # chunk-state summary matmuls first: they feed the
                        # recurrence that unblocks the NEXT chunk's inter matmuls
                        for it in range(IT):
                            for hh in range(2):
                                h, hb = 2 * it + hh, hh * 64
                                nc.tensor.matmul(
                                    s_half[h // 6][:, ts(h % 6, 64)], bw_cs[c][:, ts(h, 32)],
                                    xhs[c][it][:, hb:hb + 64],
                                    start=True, stop=True, skip_group_check=True,
                                )

                    # state recurrence: hnew = exp(s_L) * hprev + S
                    if not last:
                        hnew = mp.tile([32, 768], BF, name="hnew", bufs=2)
                        if c == 0:
                            for j in range(2):
                                nc.vector.tensor_copy(hnew[:, ts(j, 384)], s_half[j][:, :])
                        else:
                            for j in range(2):
                                t1 = mp.tile([32, 384], FP, name="t1")
                                nc.vector.tensor_tensor(
                                    t1[:, :].rearrange("p (h d) -> p h d", h=6),
                                    hprev_old[:, ts(j, 384)].rearrange("p (h d) -> p h d", h=6),
                                    esls[c - 1][:, j * 6:(j + 1) * 6, None].to_broadcast([32, 6, 64]),
                                    OP.mult,
                                )
                                nc.vector.tensor_tensor(
                                    hnew[:, ts(j, 384)], t1[:, :], s_half[j][:, :], OP.add,
                                )

                    # all intra matmuls first (independent of hnew), then
                    # the inter matmuls (which wait on the recurrence), then
                    # gating -- keeps the PE queue from stalling at chunk
                    # boundaries
                    yps = []
                    for it in range(IT):
                        yp = py.tile([128, 128], FP, space="PSUM", name="yp", bufs=6)
                        yps.append(yp)
                        for hh in range(2):
                            hb = hh * 64
                            nc.tensor.matmul(
                                yp[hb:hb + 64, :], xhs[c][it][:, hb:hb + 64],
                                mask_cs[c][:, ts(2 * it + hh, 128)],
                                start=True, stop=(c == 0), skip_group_check=True,
                            )
                    if c > 0:
                        for it in range(IT):
                            for hh in range(2):
                                hb = hh * 64
                                nc.tensor.matmul(
                                    yps[it][hb:hb + 64, :],
                                    hprev_old[:, it * 128 + hb:it * 128 + hb + 64],
                                    cs_cs[c - 1][:, ts(2 * it + hh, 128)],
                                    start=False, stop=True, skip_group_check=True,
                                )
                    for it in range(IT):
                        nc.vector.tensor_tensor(
                            g[it][:, ts(c, 128)], yps[it][:, :],
                            zs[it][:, ts(c, 128)], OP.mult,
                        )
                        # g^2 for the sumsq row, on the otherwise-idle ACT engine
                        nc.scalar.activation(
                            g2s[it][:, ts(c, 128)], g[it][:, ts(c, 128)], AF.Square
                        )
                    if not last:
                        hprev = hnew

            # ---------------- phase 3: sumsq + final projection ----------------
            with tc.tile_pool(name="pf", bufs=3, space="PSUM") as pf:
                pss = pf.tile([1, SEQ], FP, space="PSUM", name="pss", tag="ss", bufs=1)
                for i in range(IT):
                    nc.tensor.matmul(
                        pss[:, :], onescolb[:, :], g2s[i][:, :],
                        start=(i == 0), stop=(i == IT - 1),
                    )
                ssr = sbp.tile([1, SEQ], FP, name="ssr")
                nc.scalar.copy(ssr[:, :], pss[:, :])
                nc.scalar.dma_start(out=oss_d[:, :], in_=ssr[:, :])

                for o in range(OT):
                    po = pf.tile([128, SEQ], FP, space="PSUM", name="po", tag="fin")
                    for i in range(IT):
                        nc.tensor.matmul(
                            po[:, :], wmTs[i][:, ts(o, 128)], g[i][:, :],
                            start=(i == 0), stop=(i == IT - 1),
                        )
                    ob = sbp.tile([128, SEQ], BF, name="ob", bufs=3)
                    if o % 2 == 0:
                        nc.vector.tensor_copy(ob[:, :], po[:, :])
                    else:
                        nc.scalar.copy(ob[:, :], po[:, :])
                    nc.scalar.dma_start(out=out_d[ts(o, 128), :], in_=ob[:, :])

    nc.finalize()
    return nc


def _host_prep(inputs):
    x = np.asarray(inputs["x"], np.float32)
    norm_w = np.asarray(inputs["norm_w"], np.float32)
    h = x * (1.0 / np.sqrt((x * x).mean(-1, keepdims=True) + EPS)) * norm_w

    in_maps = []
    for core in range(8):
        d, b, gh = core // 4, (core // 2) % 2, core % 2
        pfx = "fwd_" if d == 0 else "bwd_"
        Wi = np.asarray(inputs[pfx + "in_w"], np.float32)
        cw = np.asarray(inputs[pfx + "conv_w"], np.float32)
        cb = np.asarray(inputs[pfx + "conv_b"], np.float32)
        dtb = np.asarray(inputs[pfx + "dt_bias"], np.float32)
        Alog = np.asarray(inputs[pfx + "A_log"], np.float32)
        Dp = np.asarray(inputs[pfx + "D"], np.float32)
        nw = np.asarray(inputs[pfx + "norm_w"], np.float32)
        Wo = np.asarray(inputs[pfx + "out_w"], np.float32)
        Wop = np.asarray(inputs["out_proj_w"], np.float32)[:, d * 768:(d + 1) * 768]

        u = h[b] if d == 0 else np.ascontiguousarray(h[b][::-1])
        hs = slice(gh * H, (gh + 1) * H)
        cs = slice(gh * DI, (gh + 1) * DI)

        wz = Wi[cs]
        wx = Wi[D_INNER:2 * D_INNER][cs]
        wb = Wi[2 * D_INNER:2 * D_INNER + 2 * D_STATE]
        wdt = Wi[D_INNER + CONV_DIM:][hs]

        cw_s = np.concatenate([cw[cs], cw[D_INNER:CONV_DIM]], 0)
        cb_s = np.concatenate([cb[cs], cb[D_INNER:CONV_DIM]], 0)

        hdr = np.zeros((128, 224), np.float32)
        hdr[:, 0:128] = np.eye(128, dtype=np.float32)
        hdr[:, 128] = 1.0
        for ct in range(NCT):
            P = 128 if ct < NCT - 1 else 64
            hdr[:P, 129 + ct] = cb_s[ct * 128:ct * 128 + P]
            hdr[:P, 136 + ct * D_CONV:136 + (ct + 1) * D_CONV] = cw_s[ct * 128:ct * 128 + P, :]

        # dmask[i, h*128+t] = D_h * delta(i, t): folds the D*x term into the mask
        dmask = np.zeros((128, H * 128), np.float32)
        for hh in range(H):
            dmask[np.arange(128), hh * 128 + np.arange(128)] = Dp[hs][hh]

        # ---- host dt/decay math (f64) ----
        A = -np.exp(Alog[hs].astype(np.float64))                   # [H]
        dtraw = u.astype(np.float64) @ wdt.T.astype(np.float64) + dtb[hs]  # [512, H]
        dt1 = np.logaddexp(0.0, dtraw)                             # softplus
        dtc = dt1.reshape(NCHUNK, LC, H)
        cloc = np.cumsum(dtc, axis=1)                              # [C, LC, H]
        s = cloc * A[None, None, :]                                # [C, LC, H]
        # me[c, i, h*128+t] = exp(min(s_t - s_i, 0)) * dt_i * causal(i <= t)
        diff = s[:, None, :, :] - s[:, :, None, :]                 # [C, i, t, H]
        me = np.exp(np.minimum(diff, 0.0)) * dtc[:, :, None, :]    # [C, i, t, H]
        me *= np.tril(np.ones((LC, LC), np.float64))[None, :, :, None]
        me = np.transpose(me, (1, 0, 3, 2)).reshape(128, NCHUNK * H * LC)
        # esc[n, (c-1)*1536 + h*128+t] = exp(s_t) (replicated over n)
        est = np.exp(np.transpose(s[1:], (0, 2, 1)))               # [C-1, H, LC]
        esc = np.broadcast_to(
            est.reshape(NCHUNK - 1, 1, H, LC), (NCHUNK - 1, 32, H, LC)
        )
        esc = np.transpose(esc, (1, 0, 2, 3)).reshape(32, (NCHUNK - 1) * H * LC)
        # wt[c, i, h] = dt_i * exp(s_L - s_i)
        wt = dtc * np.exp(s[:, -1:, :] - s)                        # [C, LC, H]
        for c in range(NCHUNK - 1):
            hdr[:, 164 + c * H:164 + (c + 1) * H] = wt[c]
        # esl[c, n, h] = exp(s_L) of chunk c (rows 0:32 replicated)
        esl_v = np.exp(s[:, -1, :])                                # [C, H]
        for c in range(1, NCHUNK - 1):
            hdr[0:32, 200 + (c - 1) * H:200 + c * H] = esl_v[c][None, :]

        Wm = (Wop @ Wo) * nw[None, :]
        WmT = Wm[:, cs].T                                          # [DI, D_MODEL]

        # packed per-k phase-1 weights: [uT_k | wxbcT_k | wzT_k]
        uT = u.T                                                   # [768, 512]
        wxbcT = np.concatenate([wx, wb], 0).T                      # [768, 832]
        wzT = wz.T                                                 # [768, 768]
        pka = np.zeros((KT, 128, PKA), NPBF)
        pkb = np.zeros((KT, 128, DI), NPBF)
        for k in range(KT):
            sl = slice(k * 128, (k + 1) * 128)
            pka[k, :, 0:SEQ] = uT[sl].astype(NPBF)
            pka[k, :, SEQ:SEQ + XBC] = wxbcT[sl].astype(NPBF)
            pkb[k, :, :] = wzT[sl].astype(NPBF)

        wm = np.zeros((128, KT * D_MODEL), NPBF)
        for k in range(KT):
            wm[:, k * D_MODEL:(k + 1) * D_MODEL] = WmT[k * 128:(k + 1) * 128].astype(NPBF)

        me = np.concatenate([me, dmask], axis=1)
        m = dict(
            hd=hdr.astype(NPBF),
            pka=pka,
            pkb=pkb,
            me=me.astype(NPBF),
            esc=np.ascontiguousarray(esc).astype(NPBF),
            wm=wm,
        )
        in_maps.append(m)
    return in_maps, h, x


_NC_CACHE = {}


def run_cores(in_maps, trace=False, tmpdir=None):
    from concourse.bass_utils import run_bass_kernel_spmd

    if "nc" not in _NC_CACHE:
        _NC_CACHE["nc"] = build_nc()
    nc = _NC_CACHE["nc"]
    return run_bass_kernel_spmd(
        nc, in_maps, core_ids=list(range(8)), trace=trace, tmpdir=tmpdir
    )


def combine(results, x):
    out = x.copy()
    for d in range(2):
        for b in range(2):
            q0, q1 = results[d * 4 + b * 2 + 0], results[d * 4 + b * 2 + 1]
            r0 = np.asarray(q0["out"], np.float32)
            r1 = np.asarray(q1["out"], np.float32)
            P = (r0 + r1).T
            sstot = np.asarray(q0["oss"], np.float32)[0] + np.asarray(q1["oss"], np.float32)[0]
            r = 1.0 / np.sqrt(sstot / D_INNER + EPS)
            y = P * r[:, None]
            out[b] += y[::-1] if d == 1 else y
    return out


def kernel(**inputs):
    in_maps, h, x = _host_prep(inputs)
    res = run_cores(in_maps).results
    return combine(res, x)


if __name__ == "__main__":
    import reference

    inputs = {k: np.asarray(v) for k, v in reference.setup_inputs().items()}
    out = kernel(**inputs)
    print("out", out.shape, out.dtype)


# revision 36
# speedup vs baseline: 1.0104x; 1.0104x over previous
"""BiMamba block Trainium2 kernel (8 NeuronCores, communication-free sharding).

Sharding: 8 cores = 2 directions x 2 batches x 2 head-halves (12 of 24 Mamba2
heads per core).  Per core: in_proj slice -> causal depthwise conv (diagonal
matmuls) -> chunked SSD scan (chunk=128) -> gating -> partial out-projection
with the merged (out_proj @ inner_out_proj * norm_w) weight.  The gated
RMSNorm's row scaling commutes with the final matmul, so each core returns an
unnormalized partial [768, 512] plus a per-token sum-of-squares row; the host
applies rsqrt(mean+eps), sums partials, reverses the backward direction and
adds the residual.  No inter-core communication.

The dt -> softplus -> cumsum -> exp decay math (0.06 GFLOP) is precomputed on
the host in f64 and shipped as bf16 decay masks; all device matmuls are bf16
with f32 PSUM accumulation.

Perf notes (vs the first working version):
 - inputs are packed into a handful of DRAM params so the DMA queue issues
   ~12 large transfers instead of 40 small ones
 - all 24 x^T chunk tiles are produced ahead of the scan: chunk 0 via PE
   transposes (lowest latency), chunks 1-3 via DMA transposes split across
   the two HWDGE queues (sync + scalar), all overlapped with phase 1
 - decay masks / chunk-state scalings for every chunk are prebuilt before
   the scan so the scan's critical path is matmuls + gating only
 - the conv processes the B/C tile first so the G2 masks and B^T tiles are
   ready early
 - g^2 for the row sum-of-squares runs on DVE, not the Activation engine
 - a dummy Silu at kernel start pre-loads the activation table
"""

import sys

sys.path.insert(0, "/opt/trn_rl_repo")

import ml_dtypes
import numpy as np

import concourse.bacc as bacc
import concourse.bass as bass
import concourse.mybir as mybir
from concourse.tile import TileContext

FP = mybir.dt.float32
BF = mybir.dt.bfloat16
NPBF = ml_dtypes.bfloat16

D_MODEL = 768
D_STATE = 32
D_CONV = 4
D_INNER = 1536
HEADDIM = 64
CONV_DIM = D_INNER + 2 * D_STATE  # 1600
B_SZ, SEQ = 2, 512
EPS = 1e-5

H = 12                      # heads per core
DI = H * HEADDIM            # 768 d_inner slice per core
XBC = DI + 2 * D_STATE      # 832 conv channels per core
NCT = 7                     # conv channel tiles (6x128 + 1x64)
LC = 128                    # chunk length
NCHUNK = SEQ // LC          # 4
KT = D_MODEL // 128         # 6 k tiles
IT = DI // 128              # 6 d_inner tiles per core
OT = D_MODEL // 128         # 6 output tiles

PKA = SEQ + XBC             # 1344 packed cols per k tile (uT | wxbcT)

AF = mybir.ActivationFunctionType
OP = mybir.AluOpType


def build_nc():
    nc = bacc.Bacc(target_bir_lowering=False)

    hd_d = nc.declare_dram_parameter("hd", [128, 224], BF, isOutput=False)

    pka_d = nc.declare_dram_parameter("pka", [KT, 128, PKA], BF, isOutput=False)
    pkb_d = nc.declare_dram_parameter("pkb", [KT, 128, DI], BF, isOutput=False)
    me_d = nc.declare_dram_parameter("me", [128, (NCHUNK + 1) * H * 128], BF, isOutput=False)
    esc_d = nc.declare_dram_parameter("esc", [32, (NCHUNK - 1) * H * 128], BF, isOutput=False)
    wm_d = nc.declare_dram_parameter("wm", [128, KT * D_MODEL], BF, isOutput=False)
    out_d = nc.declare_dram_parameter("out", [D_MODEL, SEQ], BF, isOutput=True)
    oss_d = nc.declare_dram_parameter("oss", [1, SEQ], FP, isOutput=True)

    ts = bass.ts

    with TileContext(nc) as tc:
        with (
            tc.tile_pool(name="wp", bufs=1) as wp,        # weights + consts
            tc.tile_pool(name="sb", bufs=1) as sbp,       # long-lived activations
        ):
            # activation-table warmup: a dummy Silu with no data deps so the
            # ACT table load overlaps the input DMAs instead of blocking conv
            warm = wp.tile([1, 3], FP, name="warm")
            nc.vector.memset(warm[:, 0:1], 0.0)
            nc.scalar.activation(warm[:, 1:2], warm[:, 0:1], AF.Silu)
            nc.scalar.activation(warm[:, 2:3], warm[:, 0:1], AF.Square)

            pka0 = wp.tile_from(pka_d[0, :, :], name="pka0")
            hdr = wp.tile_from(hd_d[:, :], name="hdr")
            identb = hdr[:, 0:128]
            onescolb = hdr[:, 128:129]
            # tensor_scalar / activation-bias operands must be f32: one cast
            cwf = wp.tile([128, 35], FP, name="cwf")
            nc.vector.tensor_copy(cwf[:, :], hdr[:, 129:164])
            convbs = [cwf[:, ct:ct + 1] for ct in range(NCT)]
            convws = [cwf[:, 7 + ct * D_CONV:7 + (ct + 1) * D_CONV] for ct in range(NCT)]
            wts = [hdr[:, 164 + c * H:164 + (c + 1) * H] for c in range(NCHUNK - 1)]
            esls = [hdr[0:32, 200 + (c - 1) * H:200 + c * H] for c in range(1, NCHUNK - 1)]

            pkas = [pka0] + [wp.tile_from(pka_d[k, :, :], name=f"pka{k}") for k in range(1, KT)]
            pkbs = [wp.tile_from(pkb_d[k, :, :], name=f"pkb{k}") for k in range(KT)]
            uTs = [pkas[k][:, 0:SEQ] for k in range(KT)]
            wxbcTs = [pkas[k][:, SEQ:SEQ + XBC] for k in range(KT)]
            wzTs = [pkbs[k][:, :] for k in range(KT)]

            me_t = wp.tile_from(me_d[:, :], name="me")
            dmask = me_t[:, NCHUNK * H * 128:]
            esc_t = wp.tile_from(esc_d[:, :], name="esc")
            wm_t = wp.tile_from(wm_d[:, :], name="wm")
            mes = [me_t[:, c * H * 128:(c + 1) * H * 128] for c in range(NCHUNK)]
            escs = [esc_t[:, (c - 1) * H * 128:c * H * 128] for c in range(1, NCHUNK)]
            wmTs = [wm_t[:, k * D_MODEL:(k + 1) * D_MODEL] for k in range(KT)]

            # conv diagonal matrices, built on DVE (Pool stalls DVE via the
            # shared SBUF ports, so it gets no tensor work)
            convds = []
            for ct in range(NCT):
                P = 128 if ct < NCT - 1 else 64
                row = []
                for k in range(D_CONV):
                    cd = wp.tile([128, 128], BF, name=f"cd{ct}_{k}")
                    nc.vector.tensor_scalar(
                        cd[:P, :P], identb[:P, :P], convws[ct][:P, k:k + 1], None, OP.mult
                    )
                    row.append(cd)
                convds.append(row)

            # long-lived SBUF activations
            zs = [sbp.tile([128, SEQ], BF, name=f"zs{i}") for i in range(IT)]
            xc = [sbp.tile([128, SEQ], BF, name=f"xc{i}") for i in range(NCT - 1)]
            bct = sbp.tile([64, SEQ], BF, name="bct")      # conv'd B(0:32) C(32:64)
            ct_sb = sbp.tile([32, SEQ], BF, name="ct_sb")  # C rows re-based to partition 0
            g = [sbp.tile([128, SEQ], BF, name=f"g{i}") for i in range(IT)]
            g2s = [sbp.tile([128, SEQ], BF, name=f"gg{i}") for i in range(IT)]
            bts = [sbp.tile([128, 32], BF, name=f"bt_{c}") for c in range(NCHUNK - 1)]
            g2cs = [sbp.tile([128, 128], BF, name=f"g2c_{c}") for c in range(NCHUNK)]
            # x^T tiles for the scan: xh[c][it] = transpose(xc[it][:, chunk c])
            xhs = [[sbp.tile([128, 128], BF, name=f"xh{c}_{i}") for i in range(IT)]
                   for c in range(NCHUNK)]
            # prebuilt scan tensors
            mask_cs = [sbp.tile([128, H * 128], BF, name=f"mask{c}") for c in range(NCHUNK)]
            cs_cs = [sbp.tile([32, H * 128], BF, name=f"cs{c}") for c in range(1, NCHUNK)]
            bw_cs = [sbp.tile([128, H * 32], BF, name=f"bw{c}") for c in range(NCHUNK - 1)]

            # ---------------- phase 1: in_proj + conv ----------------
            with (
                tc.tile_pool(name="pbig", bufs=2, space="PSUM") as pbig,
                tc.tile_pool(name="psmall", bufs=3, space="PSUM") as psmall,
            ):
                cin = [None] * NCT
                job_groups = (
                    [[("x", 6), ("x", 0), ("x", 1), ("x", 2), ("x", 3)],
                     [("x", 4), ("x", 5)]]
                    + [[("z", i) for i in range(5)], [("z", 5)]]
                )

                def do_conv():
                    # B/C tile (6) first: it unblocks G2 / B^T / cs_c early
                    for ctile in [NCT - 1] + list(range(NCT - 1)):
                        P = 128 if ctile < NCT - 1 else 64
                        pc = pbig.tile([128, SEQ], FP, space="PSUM", name="pc", tag="big", bufs=5)
                        for k in range(D_CONV):
                            nc.tensor.matmul(
                                pc[:P, :], convds[ctile][k][:P, :P], cin[ctile][:P, k:k + SEQ],
                                start=(k == 0), stop=(k == D_CONV - 1),
                            )
                        dst = xc[ctile][:, :] if ctile < NCT - 1 else bct[:, :]
                        nc.scalar.activation(dst, pc[:P, :], AF.Silu, bias=convbs[ctile][:P, :])
                        if ctile == NCT - 1:
                            nc.scalar.copy(ct_sb[:, :], bct[32:64, :])
                            # G2 masks (shared across heads): (B C^T) * causal
                            for c in range(NCHUNK):
                                pg = psmall.tile([128, 128], FP, space="PSUM", name="pg", tag="sm")
                                nc.tensor.matmul(
                                    pg[:, :], bct[0:32, ts(c, 128)], ct_sb[:, ts(c, 128)],
                                    start=True, stop=True,
                                )
                                nc.vector.tensor_copy(g2cs[c][:, :], pg[:, :])
                            # B^T per chunk: [32, 128] -> [128, 32]
                            for c in range(NCHUNK - 1):
                                pbt = psmall.tile([128, 32], BF, space="PSUM", name="pbt", tag="sm")
                                nc.tensor.transpose(pbt[:, :], bct[0:32, ts(c, 128)], identb[0:32, 0:32])
                                nc.vector.tensor_copy(bts[c][:, :], pbt[:, :])
                        else:
                            # x^T tiles for this conv tile, all via PE
                            # transposes (the DMA queues are busy streaming
                            # inputs; PE transposes are ~150ns each)
                            it = ctile
                            for c in range(NCHUNK):
                                ptx = psmall.tile([128, 128], BF, space="PSUM", name="ptx", tag="sm")
                                nc.tensor.transpose(ptx[:, :], xc[it][:, ts(c, 128)], identb[:, :])
                                if (it + c) % 2 == 0:
                                    nc.vector.tensor_copy(xhs[c][it][:, :], ptx[:, :])
                                else:
                                    nc.scalar.copy(xhs[c][it][:, :], ptx[:, :])

                for gi, grp in enumerate(job_groups):
                    if gi == 2:
                        do_conv()
                    ptiles = {}
                    for kind, idx in grp:
                        ptiles[(kind, idx)] = pbig.tile(
                            [128, SEQ], FP, space="PSUM", name="px", tag="big", bufs=5
                        )
                    for k in range(KT):
                        for kind, idx in grp:
                            if kind == "x":
                                P = 128 if idx < NCT - 1 else 64
                                nc.tensor.matmul(
                                    ptiles[(kind, idx)][:P, :],
                                    wxbcTs[k][:, idx * 128:idx * 128 + P], uTs[k][:, :],
                                    start=(k == 0), stop=(k == KT - 1),
                                )
                            else:
                                nc.tensor.matmul(
                                    ptiles[(kind, idx)][:, :],
                                    wzTs[k][:, ts(idx, 128)], uTs[k][:, :],
                                    start=(k == 0), stop=(k == KT - 1),
                                )
                    for kind, idx in grp:
                        if kind == "x":
                            P = 128 if idx < NCT - 1 else 64
                            ci = sbp.tile([128, D_CONV - 1 + SEQ], BF, name=f"cin{idx}")
                            nc.vector.memset(ci[:P, 0:D_CONV - 1], 0.0)
                            nc.vector.tensor_copy(ci[:P, D_CONV - 1:], ptiles[(kind, idx)][:P, :])
                            cin[idx] = ci
                        else:
                            nc.scalar.activation(
                                zs[idx][:, :], ptiles[(kind, idx)][:, :], AF.Silu
                            )

                # prebuild every chunk's scan tensors (DVE) so the scan loop
                # is matmuls + gating only
                for c in range(NCHUNK):
                    nc.vector.tensor_tensor(
                        mask_cs[c][:, :].rearrange("p (h t) -> p h t", h=H),
                        mes[c].rearrange("p (h t) -> p h t", h=H),
                        g2cs[c][:, None, :].to_broadcast([128, H, 128]),
                        OP.mult,
                    )
                    nc.vector.tensor_tensor(mask_cs[c][:, :], mask_cs[c][:, :], dmask[:, :], OP.add)
                    if c > 0:
                        nc.vector.tensor_tensor(
                            cs_cs[c - 1][:, :].rearrange("p (h t) -> p h t", h=H),
                            escs[c - 1].rearrange("p (h t) -> p h t", h=H),
                            ct_sb[:, None, ts(c, 128)].to_broadcast([32, H, 128]),
                            OP.mult,
                        )
                    if c < NCHUNK - 1:
                        nc.vector.tensor_tensor(
                            bw_cs[c][:, :].rearrange("p (h n) -> p h n", h=H),
                            bts[c][:, None, :].to_broadcast([128, H, 32]),
                            wts[c][:, :, None].to_broadcast([128, H, 32]),
                            OP.mult,
                        )

            # ---------------- phase 2: chunked scan ----------------
            with (
                tc.tile_pool(name="py", bufs=4, space="PSUM") as py,
                tc.tile_pool(name="ps", bufs=1, space="PSUM") as psst,
                tc.tile_pool(name="mp", bufs=3) as mp,
            ):
                hprev = None
                for c in range(NCHUNK):
                    hprev_old = hprev
                    last = c == NCHUNK - 1

                    if not last:
                        s_half = [psst.tile([32, 384], FP, space="PSUM", name=f"sh{j}") for j in range(2)]
                        # chunk-state summary matmuls first: they feed the
                        # recurrence that unblocks the NEXT chunk's inter matmuls
                        for it in range(IT):
                            for hh in range(2):
                                h, hb = 2 * it + hh, hh * 64
                                nc.tensor.matmul(
                                    s_half[h // 6][:, ts(h % 6, 64)], bw_cs[c][:, ts(h, 32)],
                                    xhs[c][it][:, hb:hb + 64],
                                    start=True, stop=True, skip_group_check=True,
                                )

                    # state recurrence: hnew = exp(s_L) * hprev + S
                    if not last:
                        hnew = mp.tile([32, 768], BF, name="hnew", bufs=2)
                        if c == 0:
                            for j in range(2):
                                nc.vector.tensor_copy(hnew[:, ts(j, 384)], s_half[j][:, :])
                        else:
                            for j in range(2):
                                t1 = mp.tile([32, 384], FP, name="t1")
                                nc.vector.tensor_tensor(
                                    t1[:, :].rearrange("p (h d) -> p h d", h=6),
                                    hprev_old[:, ts(j, 384)].rearrange("p (h d) -> p h d", h=6),
                                    esls[c - 1][:, j * 6:(j + 1) * 6, None].to_broadcast([32, 6, 64]),
                                    OP.mult,
                                )
                                nc.vector.tensor_tensor(
                                    hnew[:, ts(j, 384)], t1[:, :], s_half[j][:, :], OP.add,
                                )

                    # all intra matmuls first (independent of hnew), then
                    # the inter matmuls (which wait on the recurrence), then
                    # gating -- keeps the PE queue from stalling at chunk
                    # boundaries
                    yps = []
                    for it in range(IT):
                        yp = py.tile([128, 128], FP, space="PSUM", name="yp", bufs=6)
                        yps.append(yp)
                        for hh in range(2):
                            hb = hh * 64
                            nc.tensor.matmul(
                                yp[hb:hb + 64, :], xhs[c][it][:, hb:hb + 64],
                                mask_cs[c][:, ts(2 * it + hh, 128)],
                                start=True, stop=(c == 0), skip_group_check=True,
                            )
                    if c > 0:
                        for it in range(IT):
                            for hh in range(2):
                                hb = hh * 64
                                nc.tensor.matmul(
                                    yps[it][hb:hb + 64, :],
                                    hprev_old[:, it * 128 + hb:it * 128 + hb + 64],
                                    cs_cs[c - 1][:, ts(2 * it + hh, 128)],
                                    start=False, stop=True, skip_group_check=True,
                                )
                    for it in range(IT):
                        nc.vector.tensor_tensor(
                            g[it][:, ts(c, 128)], yps[it][:, :],
                            zs[it][:, ts(c, 128)], OP.mult,
                        )
                        # g^2 for the sumsq row, on the otherwise-idle ACT engine
                        nc.scalar.activation(
                            g2s[it][:, ts(c, 128)], g[it][:, ts(c, 128)], AF.Square
                        )
                    if not last:
                        hprev = hnew

            # ---------------- phase 3: sumsq + final projection ----------------
            with tc.tile_pool(name="pf", bufs=3, space="PSUM") as pf:
                pss = pf.tile([1, SEQ], FP, space="PSUM", name="pss", tag="ss", bufs=1)
                for i in range(IT):
                    nc.tensor.matmul(
                        pss[:, :], onescolb[:, :], g2s[i][:, :],
                        start=(i == 0), stop=(i == IT - 1),
                    )
                ssr = sbp.tile([1, SEQ], FP, name="ssr")
                nc.scalar.copy(ssr[:, :], pss[:, :])
                nc.scalar.dma_start(out=oss_d[:, :], in_=ssr[:, :])

                for o in range(OT):
                    po = pf.tile([128, SEQ], FP, space="PSUM", name="po", tag="fin")
                    for i in range(IT):
                        nc.tensor.matmul(
                            po[:, :], wmTs[i][:, ts(o, 128)], g[i][:, :],
                            start=(i == 0), stop=(i == IT - 1),
                        )
                    ob = sbp.tile([128, SEQ], BF, name="ob", bufs=3)
                    if o % 2 == 0:
                        nc.vector.tensor_copy(ob[:, :], po[:, :])
                    else:
                        nc.scalar.copy(ob[:, :], po[:, :])
                    nc.scalar.dma_start(out=out_d[ts(o, 128), :], in_=ob[:, :])

    nc.finalize()
    return nc


def _host_prep(inputs):
    x = np.asarray(inputs["x"], np.float32)
    norm_w = np.asarray(inputs["norm_w"], np.float32)
    h = x * (1.0 / np.sqrt((x * x).mean(-1, keepdims=True) + EPS)) * norm_w

    in_maps = []
    for core in range(8):
        d, b, gh = core // 4, (core // 2) % 2, core % 2
        pfx = "fwd_" if d == 0 else "bwd_"
        Wi = np.asarray(inputs[pfx + "in_w"], np.float32)
        cw = np.asarray(inputs[pfx + "conv_w"], np.float32)
        cb = np.asarray(inputs[pfx + "conv_b"], np.float32)
        dtb = np.asarray(inputs[pfx + "dt_bias"], np.float32)
        Alog = np.asarray(inputs[pfx + "A_log"], np.float32)
        Dp = np.asarray(inputs[pfx + "D"], np.float32)
        nw = np.asarray(inputs[pfx + "norm_w"], np.float32)
        Wo = np.asarray(inputs[pfx + "out_w"], np.float32)
        Wop = np.asarray(inputs["out_proj_w"], np.float32)[:, d * 768:(d + 1) * 768]

        u = h[b] if d == 0 else np.ascontiguousarray(h[b][::-1])
        hs = slice(gh * H, (gh + 1) * H)
        cs = slice(gh * DI, (gh + 1) * DI)

        wz = Wi[cs]
        wx = Wi[D_INNER:2 * D_INNER][cs]
        wb = Wi[2 * D_INNER:2 * D_INNER + 2 * D_STATE]
        wdt = Wi[D_INNER + CONV_DIM:][hs]

        cw_s = np.concatenate([cw[cs], cw[D_INNER:CONV_DIM]], 0)
        cb_s = np.concatenate([cb[cs], cb[D_INNER:CONV_DIM]], 0)

        hdr = np.zeros((128, 224), np.float32)
        hdr[:, 0:128] = np.eye(128, dtype=np.float32)
        hdr[:, 128] = 1.0
        for ct in range(NCT):
            P = 128 if ct < NCT - 1 else 64
            hdr[:P, 129 + ct] = cb_s[ct * 128:ct * 128 + P]
            hdr[:P, 136 + ct * D_CONV:136 + (ct + 1) * D_CONV] = cw_s[ct * 128:ct * 128 + P, :]

        # dmask[i, h*128+t] = D_h * delta(i, t): folds the D*x term into the mask
        dmask = np.zeros((128, H * 128), np.float32)
        for hh in range(H):
            dmask[np.arange(128), hh * 128 + np.arange(128)] = Dp[hs][hh]

        # ---- host dt/decay math (f64) ----
        A = -np.exp(Alog[hs].astype(np.float64))                   # [H]
        dtraw = u.astype(np.float64) @ wdt.T.astype(np.float64) + dtb[hs]  # [512, H]
        dt1 = np.logaddexp(0.0, dtraw)                             # softplus
        dtc = dt1.reshape(NCHUNK, LC, H)
        cloc = np.cumsum(dtc, axis=1)                              # [C, LC, H]
        s = cloc * A[None, None, :]                                # [C, LC, H]
        # me[c, i, h*128+t] = exp(min(s_t - s_i, 0)) * dt_i * causal(i <= t)
        diff = s[:, None, :, :] - s[:, :, None, :]                 # [C, i, t, H]
        me = np.exp(np.minimum(diff, 0.0)) * dtc[:, :, None, :]    # [C, i, t, H]
        me *= np.tril(np.ones((LC, LC), np.float64))[None, :, :, None]
        me = np.transpose(me, (1, 0, 3, 2)).reshape(128, NCHUNK * H * LC)
        # esc[n, (c-1)*1536 + h*128+t] = exp(s_t) (replicated over n)
        est = np.exp(np.transpose(s[1:], (0, 2, 1)))               # [C-1, H, LC]
        esc = np.broadcast_to(
            est.reshape(NCHUNK - 1, 1, H, LC), (NCHUNK - 1, 32, H, LC)
        )
        esc = np.transpose(esc, (1, 0, 2, 3)).reshape(32, (NCHUNK - 1) * H * LC)
        # wt[c, i, h] = dt_i * exp(s_L - s_i)
        wt = dtc * np.exp(s[:, -1:, :] - s)                        # [C, LC, H]
        for c in range(NCHUNK - 1):
            hdr[:, 164 + c * H:164 + (c + 1) * H] = wt[c]
        # esl[c, n, h] = exp(s_L) of chunk c (rows 0:32 replicated)
        esl_v = np.exp(s[:, -1, :])                                # [C, H]
        for c in range(1, NCHUNK - 1):
            hdr[0:32, 200 + (c - 1) * H:200 + c * H] = esl_v[c][None, :]

        Wm = (Wop @ Wo) * nw[None, :]
        WmT = Wm[:, cs].T                                          # [DI, D_MODEL]

        # packed per-k phase-1 weights: [uT_k | wxbcT_k | wzT_k]
        uT = u.T                                                   # [768, 512]
        wxbcT = np.concatenate([wx, wb], 0).T                      # [768, 832]
        wzT = wz.T                                                 # [768, 768]
        pka = np.zeros((KT, 128, PKA), NPBF)
        pkb = np.zeros((KT, 128, DI), NPBF)
        for k in range(KT):
            sl = slice(k * 128, (k + 1) * 128)
            pka[k, :, 0:SEQ] = uT[sl].astype(NPBF)
            pka[k, :, SEQ:SEQ + XBC] = wxbcT[sl].astype(NPBF)
            pkb[k, :, :] = wzT[sl].astype(NPBF)

        wm = np.zeros((128, KT * D_MODEL), NPBF)
        for k in range(KT):
            wm[:, k * D_MODEL:(k + 1) * D_MODEL] = WmT[k * 128:(k + 1) * 128].astype(NPBF)

        me = np.concatenate([me, dmask], axis=1)
        m = dict(
            hd=hdr.astype(NPBF),
            pka=pka,
            pkb=pkb,
            me=me.astype(NPBF),
            esc=np.ascontiguousarray(esc).astype(NPBF),
            wm=wm,
        )
        in_maps.append(m)
    return in_maps, h, x


_NC_CACHE = {}


def run_cores(in_maps, trace=False, tmpdir=None):
    from concourse.bass_utils import run_bass_kernel_spmd

    if "nc" not in _NC_CACHE:
        _NC_CACHE["nc"] = build_nc()
    nc = _NC_CACHE["nc"]
    return run_bass_kernel_spmd(
        nc, in_maps, core_ids=list(range(8)), trace=trace, tmpdir=tmpdir
    )


def combine(results, x):
    out = x.copy()
    for d in range(2):
        for b in range(2):
            q0, q1 = results[d * 4 + b * 2 + 0], results[d * 4 + b * 2 + 1]
            r0 = np.asarray(q0["out"], np.float32)
            r1 = np.asarray(q1["out"], np.float32)
            P = (r0 + r1).T
            sstot = np.asarray(q0["oss"], np.float32)[0] + np.asarray(q1["oss"], np.float32)[0]
            r = 1.0 / np.sqrt(sstot / D_INNER + EPS)
            y = P * r[:, None]
            out[b] += y[::-1] if d == 1 else y
    return out


def kernel(**inputs):
    in_maps, h, x = _host_prep(inputs)
    res = run_cores(in_maps).results
    return combine(res, x)


if __name__ == "__main__":
    import reference

    inputs = {k: np.asarray(v) for k, v in reference.setup_inputs().items()}
    out = kernel(**inputs)
    print("out", out.shape, out.dtype)
